# revision 1
# baseline (speedup 1.0000x reference)
"""Bahdanau additive-attention kernel for Trainium2, data-parallel over
batch across 8 NeuronCores.

Per batch b:
    energy  = tanh(dec_proj[b] + enc[b] @ W_enc + b_score)   # (L, DEC)
    scores  = energy @ v                                     # (L,)
    alpha   = softmax(scores)
    att[b]  = alpha @ enc[b]                                 # (2E,)

On-device layout (per core, 8 batches):
  - enc is staged host-side as enc_t[b, e, l] so encT tiles [128e, 512l]
    feed the PE directly as the moving operand; W_enc (e, d) is the
    stationary operand in its natural layout. The matmul computes
    enc_projT (d on partitions), which makes dec_proj + b_score a
    per-partition bias fused into the ACT tanh.
  - scores = v . energyT via PE matvec accumulated over d-tiles.
  - softmax skips the max-subtraction: |scores| <= sum|v| = 32, safely
    inside the fp32 exp range, and softmax is shift-invariant. Raw
    scores are broadcast to 128 partitions with a K=1 ones-matmul, the
    Exp runs on the broadcast tile with accum_out giving the partial
    denominator replicated per partition, so the final 1/den needs no
    cross-partition traffic.
  - att^T accumulates per chunk with DVE multiply+reduce over the encT
    tiles still resident in SBUF (no second DMA pass); each chunk's
    accumulation overlaps the next chunk's matmuls.
  - startup is DMA-paced, so batch 0 chunk 0 runs k-major (4 psum
    groups per pass) with the dec_proj preamble matmuls interleaved in
    data-arrival order.
  - f32r (fp32 data, full-rate PE mode) is used for all N>=256 matmuls.
"""

import numpy as np
from contextlib import ExitStack

import concourse.bass as bass
import concourse.tile as tile
from concourse import mybir
from concourse.bass_utils import run_bass_kernel_spmd
from concourse.vector_clock import ScopedClock, VectorClock

N_CORES = 8
B, L, DEC, ENC2 = 64, 1024, 1024, 2048
BL = B // N_CORES  # batches per core
KT = ENC2 // 128   # contraction tiles over e
DT = DEC // 128    # d tiles
LC = 512           # l-chunk (one PSUM bank of f32)
NLC = L // LC

F32 = mybir.dt.float32
F32R = mybir.dt.float32r
AF = mybir.ActivationFunctionType
ALU = mybir.AluOpType
AX = mybir.AxisListType


def _patch_tile_drain():
    """Workarounds for this container's walrus build.

    1. The Tile tail drain carries one sem wait per touched proc; walrus
       rejects >2 on the CTRL encoding. Split the waits onto single-wait
       SP nops (SP executes in order, so the drain then needs none).
    2. Any instruction with 2+ sem waits can fail codegen (the matmul
       LW encoding holds a single wait). Split multi-wait instructions:
       excess waits move onto same-engine InstNoOp carriers inserted
       just before; engine program order makes this equivalent.
    """
    if getattr(tile.TileContext, "_drain_patched", False):
        return

    def _drain_and_barrier(self, tick_clock, wait_clock):
        vec = list(tick_clock.global_clock)
        n = len(vec)
        for i in range(n):
            if vec[i] <= 0:
                continue
            part = [0] * n
            part[i] = vec[i]
            nop_inst = self.nc.sync.nop(nofuse=True)
            wait_clock.add_sem_waits(
                nop_inst.ins, ScopedClock({None: VectorClock(part)})
            )
        self.nc.sync.drain()
        self.nc.all_engine_barrier()
        assert self.sems is not None
        popped = self.nc._tile_sem_poison_stack.pop()
        assert popped is self._sem_poison
        self.nc.clear_and_free_semaphores(list(self.sems.allocated().values()))
        self.nc.all_engine_barrier()

    tile.TileContext._drain_and_barrier = _drain_and_barrier

    import bass_rust

    orig_lower = tile.TileContext._lower_ordered_insts

    def _lower_with_wait_split(self, ordered):
        for insts in ordered.values():
            expanded = []
            for inst in insts:
                si = inst.sync_info
                waits = list(si.on_wait) if si and si.on_wait else []
                if len(waits) > 1:
                    for w in waits[:-1]:
                        nop = mybir.InstNoOp(
                            name=self.nc.get_next_instruction_name(),
                            engine=inst.engine,
                            bass_nofuse=True,
                            sync_info=bass_rust.SyncInfo(on_wait=[w], on_update=[]),
                        )
                        self.nc.register_instruction(nop)
                        expanded.append(nop)
                    inst.sync_info = bass_rust.SyncInfo(
                        on_wait=[waits[-1]],
                        on_update=list(si.on_update) if si.on_update else [],
                    )
                expanded.append(inst)
            insts[:] = expanded
        return orig_lower(self, ordered)

    tile.TileContext._lower_ordered_insts = _lower_with_wait_split
    tile.TileContext._drain_patched = True


def build_nc():
    _patch_tile_drain()
    nc = bass.Bass()
    enc_t = nc.declare_dram_parameter("enc_t", [BL, ENC2, L], F32R, isOutput=False)
    dec_kpb = nc.declare_dram_parameter("dec_kpb", [128, DT, BL], F32, isOutput=False)
    w_score = nc.declare_dram_parameter(
        "w_score", [DEC + ENC2, DEC], F32R, isOutput=False
    )
    b_mat = nc.declare_dram_parameter("b_mat", [128, DT], F32, isOutput=False)
    v_mat = nc.declare_dram_parameter("v_mat", [128, DT], F32R, isOutput=False)
    eye = nc.declare_dram_parameter("eye", [128, 128], F32, isOutput=False)
    ones = nc.declare_dram_parameter("ones", [1, 128], F32R, isOutput=False)
    att = nc.declare_dram_parameter("att", [BL, ENC2], F32, isOutput=True)

    with tile.TileContext(nc) as tc, ExitStack() as ctx:
        singles = ctx.enter_context(tc.tile_pool(name="singles", bufs=1))
        smalls = ctx.enter_context(tc.tile_pool(name="smalls", bufs=2))
        wdec_pool = ctx.enter_context(tc.tile_pool(name="wdec", bufs=32))
        enc_pool = ctx.enter_context(tc.tile_pool(name="enc", bufs=6))
        energy_pool = ctx.enter_context(tc.tile_pool(name="energy", bufs=3))
        wbc_pool = ctx.enter_context(tc.tile_pool(name="wbc", bufs=2))
        prod_pool = ctx.enter_context(tc.tile_pool(name="prod", bufs=2))
        ep_ps = ctx.enter_context(tc.tile_pool(name="ep_ps", bufs=4, space="PSUM"))
        sc_ps = ctx.enter_context(tc.tile_pool(name="sc_ps", bufs=1, space="PSUM"))
        wb_ps = ctx.enter_context(tc.tile_pool(name="wb_ps", bufs=1, space="PSUM"))
        dec_ps = ctx.enter_context(tc.tile_pool(name="dec_ps", bufs=1, space="PSUM"))
        att_ps_pool = ctx.enter_context(
            tc.tile_pool(name="att_ps", bufs=1, space="PSUM")
        )

        # ---- persistent tiles -------------------------------------------
        wenc = singles.tile([128, KT, DEC], F32R)  # W_enc, (e-tile, k) x d
        dec_sb = singles.tile([128, DT, BL], F32)
        b_sb = singles.tile([128, DT], F32)
        v_sb = singles.tile([128, DT], F32R)
        eye_sb = singles.tile([128, 128], F32)
        bias_sb = singles.tile([128, DT, BL], F32)  # dec_proj + b_score
        att_all = singles.tile([128, KT * BL], F32)  # att^T cols = b*KT+k
        ones_sb = singles.tile([1, 128], F32R)

        # ---- startup DMA, in data-arrival order -------------------------
        nc.sync.dma_start(out=dec_sb, in_=dec_kpb[:, :, :])
        nc.sync.dma_start(out=b_sb, in_=b_mat[:, :])
        nc.sync.dma_start(out=v_sb, in_=v_mat[:, :])
        nc.sync.dma_start(out=eye_sb, in_=eye[:, :])
        nc.sync.dma_start(out=ones_sb, in_=ones[:, :])

        # wd tiles dt-major: group k of the startup loop carries
        # (dt, kk) pairs [4k : 4k+4]
        wd_order = [(dt, kk) for dt in range(DT) for kk in range(DT)]
        wd_tiles = {}
        KH = KT // 2

        def alloc_chunk(nm):
            a = enc_pool.tile([128, KH, LC], F32R, tag="enc", name=f"{nm}a")
            bb = enc_pool.tile([128, KH, LC], F32R, tag="enc", name=f"{nm}b")
            return (a, bb)

        def enc_sl(ch, k, lo=0, width=LC, pair=False):
            t, kk = (ch[0], k) if k < KH else (ch[1], k - KH)
            if pair:
                return t[:, kk : kk + 2, lo : lo + width]
            return t[:, kk, lo : lo + width]

        enc00 = alloc_chunk("enc00")

        def emit_pre_mm(dt, kk, dpsum):
            nc.tensor.matmul(
                dpsum,
                lhsT=wd_tiles[(dt, kk)],
                rhs=dec_sb[:, kk, :],
                start=(kk == 0),
                stop=(kk == DT - 1),
            )

        # chunk (0,0) runs k-major over two 4-dt-group passes, with the
        # dec preamble matmuls interleaved in arrival order. Only the
        # dt<4 slice of W_dec loads inside the k-loop (keeps the per-k
        # DMA budget at the PE's consumption rate); dt>=4 loads after.
        def emit_wd_dma(dt, kk):
            wd = wdec_pool.tile([128, 128], F32, tag="wd", name=f"wd_{dt}_{kk}")
            nc.sync.dma_start(
                out=wd,
                in_=w_score[
                    kk * 128 : (kk + 1) * 128, dt * 128 : (dt + 1) * 128
                ].bitcast(F32),
            )
            wd_tiles[(dt, kk)] = wd

        def emit_pre_group(dt, dpsum):
            for kk in range(DT):
                emit_pre_mm(dt, kk, dpsum)
            nc.vector.tensor_scalar_add(
                out=bias_sb[:, dt, :], in0=dpsum, scalar1=b_sb[:, dt : dt + 1]
            )

        ps00 = {
            dt: ep_ps.tile([128, LC], F32, tag="ep", name=f"ps00_{dt}")
            for dt in range(4)
        }
        dpsum = dec_ps.tile([128, BL], F32)
        for k in range(KT):
            nc.sync.dma_start(
                out=enc_sl(enc00, k), in_=enc_t[0, k * 128 : (k + 1) * 128, 0:LC]
            )
            nc.sync.dma_start(
                out=wenc[:, k, :], in_=w_score[DEC + k * 128 : DEC + (k + 1) * 128, :]
            )
            for dt, kk in wd_order[k * 2 : (k + 1) * 2]:
                emit_wd_dma(dt, kk)
            for dt, kk in wd_order[k * 2 : (k + 1) * 2]:
                emit_pre_mm(dt, kk, dpsum)
                if kk == DT - 1:
                    nc.vector.tensor_scalar_add(
                        out=bias_sb[:, dt, :],
                        in0=dpsum,
                        scalar1=b_sb[:, dt : dt + 1],
                    )
            for dt in range(4):
                nc.tensor.matmul(
                    ps00[dt],
                    lhsT=wenc[:, k, dt * 128 : (dt + 1) * 128],
                    rhs=enc_sl(enc00, k),
                    start=(k == 0),
                    stop=(k == KT - 1),
                )
        for dt, kk in wd_order[KT * 2 :]:
            emit_wd_dma(dt, kk)

        def tanh_and_score(ps, sc, b, dt):
            energy = energy_pool.tile([128, LC], F32R, tag="energy")
            nc.scalar.activation(
                out=energy,
                in_=ps,
                func=AF.Tanh,
                bias=bias_sb[:, dt, b : b + 1],
                scale=1.0,
            )
            nc.tensor.matmul(
                sc,
                lhsT=v_sb[:, dt : dt + 1],
                rhs=energy,
                start=(dt == 0),
                stop=(dt == DT - 1),
            )

        sc00 = sc_ps.tile([1, LC], F32, tag="sc")
        for dt in range(4):
            tanh_and_score(ps00[dt], sc00, 0, dt)
        ps00b = {}
        for dt in range(4, DT):
            ps = ep_ps.tile([128, LC], F32, tag="ep", name=f"ps00b_{dt}")
            for k in range(KT):
                nc.tensor.matmul(
                    ps,
                    lhsT=wenc[:, k, dt * 128 : (dt + 1) * 128],
                    rhs=enc_sl(enc00, k),
                    start=(k == 0),
                    stop=(k == KT - 1),
                )
            ps00b[dt] = ps
        for dt in range(4, DT):
            emit_pre_group(dt, dpsum)
        for dt in range(4, DT):
            tanh_and_score(ps00b[dt], sc00, 0, dt)

        def load_chunk(b, c):
            ch = alloc_chunk(f"enc_{b}_{c}")
            for k in range(KT):
                nc.sync.dma_start(
                    out=enc_sl(ch, k),
                    in_=enc_t[b, k * 128 : (k + 1) * 128, c * LC : (c + 1) * LC],
                )
            return ch

        def kmajor_chunk(b, c, enc_tile):
            """Compute one chunk's scores consuming enc tiles in DMA
            arrival (k) order: dt 0-3 accumulate k-major across 4 psum
            groups, then dt 4-7 run dt-major at full speed."""
            ps = {
                dt: ep_ps.tile([128, LC], F32, tag="ep", name=f"km_{b}_{c}_{dt}")
                for dt in range(4)
            }
            for k in range(KT):
                for dt in range(4):
                    nc.tensor.matmul(
                        ps[dt],
                        lhsT=wenc[:, k, dt * 128 : (dt + 1) * 128],
                        rhs=enc_sl(enc_tile, k),
                        start=(k == 0),
                        stop=(k == KT - 1),
                    )
            sc = sc_ps.tile([1, LC], F32, tag="sc", name=f"km_sc_{b}_{c}")
            for dt in range(4):
                tanh_and_score(ps[dt], sc, b, dt)
            for dt in range(4, DT):
                p2 = ep_ps.tile([128, LC], F32, tag="ep", name=f"km2_{b}_{c}_{dt}")
                for k in range(KT):
                    nc.tensor.matmul(
                        p2,
                        lhsT=wenc[:, k, dt * 128 : (dt + 1) * 128],
                        rhs=enc_sl(enc_tile, k),
                        start=(k == 0),
                        stop=(k == KT - 1),
                    )
                tanh_and_score(p2, sc, b, dt)
            return sc

        def chunk_softmax_wacc(enc_tile, sc, b, first, tag, lo=0, width=LC):
            """Raw-score exp + weighted reduce for one finished (sub)chunk
            [lo, lo+width). Returns the per-partition denominator tile."""
            s_sb = smalls.tile([1, width], F32R, tag="ssb", name=f"ssb_{b}_{tag}")
            nc.scalar.copy(out=s_sb, in_=sc[:, lo : lo + width])
            wb = wb_ps.tile([128, width], F32, tag="wb", name=f"wb_{b}_{tag}")
            nc.tensor.matmul(wb, lhsT=ones_sb, rhs=s_sb, start=True, stop=True)
            w_bc = wbc_pool.tile([128, width], F32, tag="wbc", name=f"wbc_{b}_{tag}")
            den_c = smalls.tile([128, 1], F32, tag=f"den{tag}", name=f"den_{b}_{tag}")
            nc.scalar.activation(
                out=w_bc, in_=wb, func=AF.Exp, bias=0.0, scale=1.0, accum_out=den_c
            )
            atmp = None
            if not first:
                atmp = smalls.tile([128, KT], F32, tag="atmp", name=f"atmp_{b}_{tag}")
            # w_bc broadcast over a pair of k-tiles (0-stride middle dim)
            wb_pair = bass.AP(
                tensor=w_bc.tensor,
                offset=w_bc.offset,
                ap=[w_bc.ap[0], [0, 2], w_bc.ap[1]],
            )
            for k in range(0, KT, 2):
                col = b * KT + k
                prod = prod_pool.tile(
                    [128, 2, width], F32, tag="prod", name=f"prod_{b}_{tag}_{k}"
                )
                nc.vector.tensor_mul(
                    out=prod,
                    in0=enc_sl(enc_tile, k, lo, width, pair=True).bitcast(F32),
                    in1=wb_pair,
                )
                dst = att_all[:, col : col + 2] if first else atmp[:, k : k + 2]
                nc.vector.tensor_reduce(out=dst, in_=prod, axis=AX.X, op=ALU.add)
            if not first:
                cols = slice(b * KT, (b + 1) * KT)
                nc.vector.tensor_add(
                    out=att_all[:, cols], in0=att_all[:, cols], in1=atmp
                )
            return den_c

        def batch_epilogue(b, dens):
            """Normalize, transpose, and store one batch's attention row."""
            rden = smalls.tile([128, 1], F32, tag="rden")
            nc.vector.tensor_add(out=rden, in0=dens[0], in1=dens[1])
            for extra in dens[2:]:
                nc.vector.tensor_add(out=rden, in0=rden, in1=extra)
            nc.vector.reciprocal(out=rden, in_=rden)
            cols = slice(b * KT, (b + 1) * KT)
            nc.vector.tensor_scalar_mul(att_all[:, cols], att_all[:, cols], rden)
            att_bt = att_ps_pool.tile([KT, 128], F32, tag="abt")
            nc.tensor.transpose(att_bt, att_all[:, cols], eye_sb)
            att_sb = smalls.tile([KT, 128], F32, tag="asb")
            nc.vector.tensor_copy(out=att_sb, in_=att_bt)
            nc.sync.dma_start(
                out=att[b].rearrange("(k p) -> k p", p=128), in_=att_sb
            )

        # ---- main loop (chunk (0,0) already computed above) -------------
        dens = [chunk_softmax_wacc(enc00, sc00, 0, True, "0")]
        for b in range(BL):
            for c in range(NLC):
                if (b, c) == (0, 0):
                    continue
                enc_tile = load_chunk(b, c)
                if (b, c) == (BL - 1, NLC - 1):
                    # final chunk: two 256-wide halves, so most of the
                    # softmax+reduce tail overlaps the remaining matmuls
                    sc = sc_ps.tile([1, LC], F32, tag="sc")
                    for h in range(2):
                        lo, w = h * (LC // 2), LC // 2
                        for dt in range(DT):
                            ps = ep_ps.tile(
                                [128, w], F32, tag="ep", name=f"ps_f{h}_{dt}"
                            )
                            for k in range(KT):
                                nc.tensor.matmul(
                                    ps,
                                    lhsT=wenc[:, k, dt * 128 : (dt + 1) * 128],
                                    rhs=enc_sl(enc_tile, k, lo, w),
                                    start=(k == 0),
                                    stop=(k == KT - 1),
                                )
                            energy = energy_pool.tile(
                                [128, w], F32R, tag="energy", name=f"en_f{h}_{dt}"
                            )
                            nc.scalar.activation(
                                out=energy,
                                in_=ps,
                                func=AF.Tanh,
                                bias=bias_sb[:, dt, b : b + 1],
                                scale=1.0,
                            )
                            nc.tensor.matmul(
                                sc[:, lo : lo + w],
                                lhsT=v_sb[:, dt : dt + 1],
                                rhs=energy,
                                start=(dt == 0),
                                stop=(dt == DT - 1),
                            )
                        if h == 0:
                            dens.append(
                                chunk_softmax_wacc(
                                    enc_tile, sc, b, False, f"1h{h}", lo=lo, width=w
                                )
                            )
                        else:
                            # last sub-chunk: two 128-wide wacc pieces so
                            # only the final quarter's reduce is exposed
                            for q in range(2):
                                dens.append(
                                    chunk_softmax_wacc(
                                        enc_tile,
                                        sc,
                                        b,
                                        False,
                                        f"1h{h}q{q}",
                                        lo=lo + q * (w // 2),
                                        width=w // 2,
                                    )
                                )
                elif (b, c) in ((0, 1), (1, 0), (1, 1)):
                    # startup transient: consume tiles in arrival order
                    sc = kmajor_chunk(b, c, enc_tile)
                    dens.append(
                        chunk_softmax_wacc(enc_tile, sc, b, c == 0, str(c))
                    )
                else:
                    sc = sc_ps.tile([1, LC], F32, tag="sc")
                    for dt in range(DT):
                        ps = ep_ps.tile([128, LC], F32, tag="ep")
                        for k in range(KT):
                            nc.tensor.matmul(
                                ps,
                                lhsT=wenc[:, k, dt * 128 : (dt + 1) * 128],
                                rhs=enc_sl(enc_tile, k),
                                start=(k == 0),
                                stop=(k == KT - 1),
                            )
                        tanh_and_score(ps, sc, b, dt)
                    dens.append(
                        chunk_softmax_wacc(enc_tile, sc, b, c == 0, str(c))
                    )
            batch_epilogue(b, dens)
            dens = []

    return nc


def shard_inputs(dec_hidden, enc_output, W_score, b_score, v):
    """Full inputs -> per-core input maps (host-side layout staging)."""
    dec_hidden = np.ascontiguousarray(dec_hidden, dtype=np.float32)
    W_score = np.ascontiguousarray(W_score, dtype=np.float32)
    b_mat = np.ascontiguousarray(
        np.asarray(b_score, dtype=np.float32).reshape(DT, 128).T
    )
    v_mat = np.ascontiguousarray(np.asarray(v, dtype=np.float32).reshape(DT, 128).T)
    eye = np.eye(128, dtype=np.float32)

    in_maps = []
    for core in range(N_CORES):
        sl = slice(core * BL, (core + 1) * BL)
        # (L, BL, 2E) -> (BL, 2E, L)
        enc_t = np.ascontiguousarray(
            np.asarray(enc_output[:, sl, :], dtype=np.float32).transpose(1, 2, 0)
        )
        # (BL, DEC) -> [p, kt, b]
        dec_kpb = np.ascontiguousarray(
            dec_hidden[sl].T.reshape(DT, 128, BL).transpose(1, 0, 2)
        )
        in_maps.append(
            {
                "enc_t": enc_t,
                "ones": np.ones((1, 128), dtype=np.float32),
                "dec_kpb": dec_kpb,
                "w_score": W_score,
                "b_mat": b_mat,
                "v_mat": v_mat,
                "eye": eye,
            }
        )
    return in_maps


_NC_CACHE = None


def kernel(dec_hidden, enc_output, W_score, b_score, v):
    global _NC_CACHE
    if _NC_CACHE is None:
        _NC_CACHE = build_nc()
    nc = _NC_CACHE
    in_maps = shard_inputs(dec_hidden, enc_output, W_score, b_score, v)
    res = run_bass_kernel_spmd(nc, in_maps, list(range(N_CORES)))
    return np.concatenate([res.results[i]["att"] for i in range(N_CORES)], axis=0)

